# revision 1
# baseline (speedup 1.0000x reference)
"""CenterLoss kernel for Trainium2, data-parallel across 8 NeuronCores.

Math: the reference masks the full [B, C] squared-distance matrix with
one_hot(labels) and clamps to [1e-12, 1e12] before summing.  The mask keeps
only distmat[i, labels[i]]; every other entry becomes clip(0) = 1e-12.  The
kept entries are ~1024 (chi-square-like, 512 dof), so the clamp never binds
on them and the loss reduces to

    loss = ( sum_i ||x_i - c_{l_i}||^2 + B*(C-1)*1e-12 ) / B

Per core (B/8 = 2048 rows), raw bass pipeline, p-major row layout
(shard row 16*p + r lives at partition p, chunk r; r in [0,16)).

The critical path is Q7/SWDGE descriptor emission for the 2048-row center
gather (~8.5-11 ns per row, hardware floor; measured on both
indirect_dma_start and dma_gather).  16 per-chunk indirect DMAs are issued
back-to-back; everything else hides under that stream:
  - x and centers ship as bf16 (host cast): halves HBM traffic so the
    gather transfers never contend, and doubles DVE subtract throughput.
    Loss tolerance is 2e-2; bf16 rounding is zero-mean and contributes
    <0.1% after averaging over 8.4M squared terms.
  - vector/scalar work at 2-chunk granularity on shared pair semaphores.
  - acc columns 0-5 are stored early to hide the final DMA receipt.
"""

import sys
from contextlib import ExitStack

import ml_dtypes
import numpy as np

try:
    import concourse.bass  # noqa: F401
except ImportError:
    sys.path.insert(0, "/opt/trn_rl_repo")

import concourse.bass as bass
import concourse.mybir as mybir
from concourse.bacc import Bacc
from concourse.bass_utils import run_bass_kernel_spmd

B, C, D = 16384, 1000, 512
N_CORES = 8
B_SHARD = B // N_CORES  # 2048
P = 128
NCHUNK = B_SHARD // P  # 16 chunks, chunk r = rows {16p + r}
NPAIR = NCHUNK // 2  # 8 compute pairs
CLAMP_MIN = 1e-12
CLAMP_MAX = 1e12

_NC_CACHE = {}


def build_nc():
    nc = Bacc()
    f32 = mybir.dt.float32
    bf16 = mybir.dt.bfloat16
    x_d = nc.declare_dram_parameter("x", [B_SHARD, D], bf16, isOutput=False)
    lbl_d = nc.declare_dram_parameter(
        "labels", [P, NCHUNK], mybir.dt.int32, isOutput=False
    )
    cen_d = nc.declare_dram_parameter("centers", [C, D], bf16, isOutput=False)
    out_d = nc.declare_dram_parameter("out", [P, NPAIR], f32, isOutput=True)

    x_r = x_d.rearrange("(p r) d -> p r d", p=P)  # [128, 16, 512]

    with ExitStack() as ctx:
        x_sb = ctx.enter_context(nc.sbuf_tensor("x_sb", [P, NCHUNK, D], bf16))
        g_sb = ctx.enter_context(nc.sbuf_tensor("g_sb", [P, NCHUNK, D], bf16))
        diff_sb = ctx.enter_context(nc.sbuf_tensor("diff_sb", [P, 2, 2, D], bf16))
        sq_sb = ctx.enter_context(nc.sbuf_tensor("sq_sb", [P, 2, D], bf16))
        lbl_sb = ctx.enter_context(
            nc.sbuf_tensor("lbl_sb", [P, NCHUNK], mybir.dt.int32)
        )
        acc_sb = ctx.enter_context(nc.sbuf_tensor("acc_sb", [P, NPAIR], f32))

        block = ctx.enter_context(nc.Block())
        ls = ctx.enter_context(nc.semaphore("ls"))
        xs = [ctx.enter_context(nc.semaphore(f"xs{q}")) for q in range(2)]
        gs = [ctx.enter_context(nc.semaphore(f"gs{k}")) for k in range(NPAIR)]
        vs = ctx.enter_context(nc.semaphore("vs"))
        ss = ctx.enter_context(nc.semaphore("ss"))
        os_ = ctx.enter_context(nc.semaphore("os"))

        @block.sync
        def _(sync):
            # labels first: the gather stream (Q7 descriptor emission) is the
            # critical path and only needs this tiny tile
            sync.dma_start(out=lbl_sb[:], in_=lbl_d[:]).then_inc(ls, 16)
            for q in range(2):
                sync.dma_start(
                    out=x_sb[:, q * 8 : (q + 1) * 8, :],
                    in_=x_r[:, q * 8 : (q + 1) * 8, :],
                ).then_inc(xs[q], 16)

        @block.gpsimd
        def _(gpsimd):
            gpsimd.wait_ge(ls, 16)
            for r in range(NCHUNK):
                gpsimd.indirect_dma_start(
                    out=g_sb[:, r, :],
                    out_offset=None,
                    in_=cen_d[:],
                    in_offset=bass.IndirectOffsetOnAxis(
                        ap=lbl_sb[:, r : r + 1], axis=0
                    ),
                ).then_inc(gs[r // 2], 16)

        @block.vector
        def _(vector):
            for k in range(NPAIR):
                vector.wait_ge(xs[k // 4], 16)
                vector.wait_ge(gs[k], 32)  # both chunks of the pair landed
                if k >= 2:
                    vector.wait_ge(ss, k - 1)  # WAR: scalar done with diff slot
                vector.tensor_tensor(
                    out=diff_sb[:, k % 2, :, :],
                    in0=x_sb[:, 2 * k : 2 * k + 2, :],
                    in1=g_sb[:, 2 * k : 2 * k + 2, :],
                    op=mybir.AluOpType.subtract,
                ).then_inc(vs, 1)

        @block.scalar
        def _(scalar):
            for k in range(NPAIR):
                scalar.wait_ge(vs, k + 1)
                scalar.activation(
                    out=sq_sb[:, :, :],
                    in_=diff_sb[:, k % 2, :, :],
                    func=mybir.ActivationFunctionType.Square,
                    accum_out=acc_sb[:, k : k + 1],
                ).then_inc(ss, 1)
                if k == NPAIR - 3:
                    # early store of the first 6 columns hides most of the
                    # final DMA's completion receipt behind the last pairs.
                    # ss fires on ACTIVATION_READ_ACCUMULATOR completion, so
                    # this wait orders the store after the accum writes (the
                    # DMA trigger otherwise races the accumulator read-out).
                    scalar.wait_ge(ss, NPAIR - 2)
                    scalar.dma_start(
                        out=out_d[:, : NPAIR - 2], in_=acc_sb[:, : NPAIR - 2]
                    ).then_inc(os_, 16)
            scalar.wait_ge(ss, NPAIR)
            scalar.dma_start(
                out=out_d[:, NPAIR - 2 :], in_=acc_sb[:, NPAIR - 2 :]
            ).then_inc(os_, 16)
            scalar.wait_ge(os_, 32)

    nc.finalize()
    return nc


def _get_nc():
    if "nc" not in _NC_CACHE:
        _NC_CACHE["nc"] = build_nc()
    return _NC_CACHE["nc"]


def kernel(x, labels, centers, _trace=False):
    x = np.asarray(x, dtype=np.float32).astype(ml_dtypes.bfloat16)
    centers = np.asarray(centers, dtype=np.float32).astype(ml_dtypes.bfloat16)
    labels_i = np.asarray(labels).astype(np.int32)

    in_maps = []
    for i in range(N_CORES):
        xs_ = np.ascontiguousarray(x[i * B_SHARD : (i + 1) * B_SHARD])
        ls_ = labels_i[i * B_SHARD : (i + 1) * B_SHARD]
        in_maps.append(
            {
                "x": xs_,
                # row 16p + r at [p, r]
                "labels": np.ascontiguousarray(ls_.reshape(P, NCHUNK)),
                "centers": centers,
            }
        )

    nc = _get_nc()
    res = run_bass_kernel_spmd(nc, in_maps, list(range(N_CORES)), trace=_trace)
    partials = np.stack([r["out"] for r in res.results])  # [8, 128, 8]
    total = np.sum(partials.astype(np.float64))
    total += B * (C - 1) * CLAMP_MIN
    loss = np.float32(total / B)
    if _trace:
        return np.asarray(loss), res
    return np.asarray(loss)

